# revision 39
# baseline (speedup 1.0000x reference)
"""MLA prefill attention (DeepSeek-style), tensor-parallel over heads on 8 TRN2 NeuronCores.

Reference computation (per head h, per batch b of 4 x 1024 tokens):
  kv_c   = k[:, 0, :512]                  # [N, 512] compressed latent (shared)
  k_nope = kv_c @ w_key[h].T              # [N, 128]
  k_full = concat(k_nope, k_rope)         # [N, 192]
  v_raw  = kv_c @ w_vo[h].T               # [N, 128]
  o      = softmax(causal(q_h @ k_full.T * SCALE)) @ v_raw

Sharding: 16 heads / 8 cores = 2 heads per core; kv_c replicated. No collectives.

Device kernel (per core, all matmuls bf16):
  Phase 1 per 512-token block: v for both heads, two 128-token chunks packed
    per 1-bank PSUM tile (512 f32 exactly; only the bank's first matmul may
    use start=True -- it clears has_written for the WHOLE bank), evacuated by
    ONE merged DVE copy per tile; k_nopeT for both heads into one 2-bank
    PSUM tile (h-interleaved).  Diagonal score pairs that fit one bank
    (256+128) also pack side by side, cutting their exp span 640 -> 384
    cols.  ALL PSUM evacuation copies ride DVE: the
    scalar queue carries DMA-issue instructions whose semaphore gates would
    stall queued copies (and the PSUM rings behind them) for microseconds.
  Phase 2: transposed-score flash attention processed in chunk PAIRS: the two
    chunks' score matmuls land in the two banks of one [128,1024] PSUM tile,
    interleaved (n1,n2,r1,r2) to hide accumulation drains, then ONE exp
    activation covers the pair (halves ACT instruction overhead). Causal
    triangle masked multiplicatively on probs (bf16, SBUF) after exp, off the
    PSUM path. PV uses probs blocks as stationary with v_aug moving
    ([v|1] -> softmax denominator rides in column 128). Output is written
    UNNORMALIZED as bf16 [o(128) | den | pad] per token; the host divides.
  The PV software pipeline trails TWO pairs behind the scores and spans
    q-block/head boundaries, so a boundary exp straggler never drains the PE.
    At the p2(b0) -> p1(6) re-entry the two pending flushes interleave with
    p1's v-halves (flush_mid), hiding the final exp stragglers under v
    matmuls instead of a ~1us PE bubble.
  Rope matmuls are ZERO-PADDED to contract 128 (kr/qr SBUF rows 64-127
    memset to 0): the PE never leaves 128x128 tiling mode.  The 64-contract
    alternative pays ~195ns of tiling-mode-switch drains per pair (measured),
    which exceeds the extra streaming cost of the zero rows.
  PE WARM-UP: ~10 throwaway matmuls from a memset scratch tile run during
    the engine-boot + DMA ramp dead time (first ~10us), so the HAM clock
    gate is at 2.4 GHz when real data lands (throttle time 11.5us -> 7us).
  Emission order p1(blk 0..5), p2(batch0), p1(6), p1(7), p2(b1..b3) keeps the
  PE busy through the input-DMA ramp (q / later kv blocks arrive while early
  blocks compute). Inputs split across both HWDGE rings (sync + scalar) in
  arrival-priority order; each ring recycles only 4 completion semaphores,
  so the 5th+ issue WOULD block its engine queue (and with it ACT's kn
  copies) -- later dma_starts are therefore injected BETWEEN p1 block
  emissions.  All transfers are contiguous (a strided kv0 experiment ran at
  ~40 GB/s vs ~150 per ring).  Outputs ride the sync ring p-major ([h, qblk,
  p, 4*130], 1040B/partition rows, 4x bigger packets than token-major) in
  two half-DMAs per q-block, the first issued mid-q-block so the kernel-end
  drain is one 66KB transfer; the host unscrambles + divides.

Measured on 8-core axon TRN2: 98.3us (session start: 110.9us; first
session's baseline: 141us), rel fro err 4.7e-3.  exec_time is run-variable
(+-3us; occasional whole runs at 2.0 GHz P0 power state) -- compare kernels
only within one process (test_ab.py).
"""

import os
import sys

sys.path.insert(0, "/opt/trn_rl_repo")

from contextlib import ExitStack

import numpy as np
import ml_dtypes

import concourse.bass as bass
import concourse.mybir as mybir
from concourse import bacc, tile
from concourse.bass_utils import run_bass_kernel_spmd

B, S, H, N = 4, 1024, 16, 4096
DN, DR, DV, R = 128, 64, 128, 512
SCALE = 0.07216878364870323
NCORES = 8
HPC = H // NCORES  # heads per core
P = 128
QBLK = 512
NRC = R // P  # 4 r-chunks
NBLK = 8      # kv column blocks
BCOLS = N // NBLK
DVA = DV + 1    # v | ones  -> rowsums fall out of PV
DVAP = DV + 2   # pair stride padded so both PSUM slices are 8B-aligned
DVAO = DV + 2   # output stride per token: [o(128) | den | pad]
VCH = 2 * DVAP  # both-heads v chunk stride [v0 | 1 | pad | v1 | 1 | pad]
BF16 = mybir.dt.bfloat16
F32 = mybir.dt.float32
Exp = mybir.ActivationFunctionType.Exp
EXP_BIAS = -2.5  # shift-invariant softmax bias keeps exp outputs small

_CACHE: dict = {}


def _build():
    nc = bacc.Bacc("TRN2", target_bir_lowering=False, debug=False, num_devices=NCORES)

    qtn = nc.dram_tensor("qtn", [HPC, DN, N], BF16, kind="ExternalInput").ap()
    # rope q packed both heads on 64 partitions: [64, h*N + n]
    qrt = nc.dram_tensor("qrt", [DR, HPC * N], BF16, kind="ExternalInput").ap()
    # per column-block, r-chunks side by side: [blk][128r, c*BCOLS + n]
    kvt = nc.dram_tensor("kvt", [NBLK, P, NRC * BCOLS], BF16,
                         kind="ExternalInput").ap()
    krt = nc.dram_tensor("krt", [DR, N], BF16, kind="ExternalInput").ap()
    # w_key both heads: [128r(of chunk c), (h*NRC + c)*DN + d]
    wkt = nc.dram_tensor("wkt", [P, HPC * NRC * DN], BF16, kind="ExternalInput").ap()
    # w_vo both heads per r-chunk: [128r, (c*HPC + h)*DV + d]
    wvt = nc.dram_tensor("wvt", [P, NRC * HPC * DV], BF16, kind="ExternalInput").ap()
    mskd = nc.dram_tensor("mskd", [P, P], BF16, kind="ExternalInput").ap()
    # unnormalized output + denominator, p-major per q-block so each SBUF
    # partition DMAs one contiguous 1040B row (4x bigger packets than
    # token-major); host divides + unscrambles
    out = nc.dram_tensor("out", [HPC, N // QBLK, P, 4 * DVAO], BF16,
                         kind="ExternalOutput").ap()

    with tile.TileContext(nc) as tc, ExitStack() as ctx:
        const = ctx.enter_context(tc.tile_pool(name="const", bufs=1))
        res = ctx.enter_context(tc.tile_pool(name="res", bufs=1))
        prs = ctx.enter_context(tc.tile_pool(name="prs", bufs=4))
        osb = ctx.enter_context(tc.tile_pool(name="osb", bufs=4))
        psA = ctx.enter_context(tc.tile_pool(name="psA", bufs=2, space="PSUM"))
        psO = ctx.enter_context(tc.tile_pool(name="psO", bufs=4, space="PSUM"))

        ebias = const.tile([P, 1], F32)
        nc.gpsimd.memset(ebias[:], EXP_BIAS)

        # ---- PE warm-up: the HAM clock gate keeps the PE at 1.2 GHz until
        # ~3.4us of sustained matmul activity.  The first real matmul can't
        # start until ~10.5us (engine boot + DMA ramp), so burn that dead time
        # on throwaway matmuls from a memset scratch tile: the PE is at
        # 2.4 GHz the moment real data lands (saves the ~4.8us cold penalty).
        scr_w = const.tile([P, QBLK], BF16)
        nc.gpsimd.memset(scr_w[:], 0.03)
        scr_ps = psA.tile([P, 2 * QBLK], F32, tag="psA", name="warm")
        for wi in range(10):
            nc.tensor.matmul(
                scr_ps[:, (wi % 2) * QBLK:(wi % 2) * QBLK + QBLK],
                lhsT=scr_w[:, 0:P], rhs=scr_w[:],
                start=True, stop=True, skip_group_check=True,
            )

        # ---- input DMAs: need-sorted, alternating between the two HWDGE
        # rings (sync + scalar) so the rings' packet-granular round-robin over
        # the 16 SDMA engines reproduces a single priority-ordered stream at
        # full bandwidth.  Need order: kv0a/wv (first v group), kv0b/wk
        # (second group + k_nope), kv1..kv5, then q/kr for p2(b0) split so the
        # batch-0 slices land first, then kv6/kv7 (p1 resumes after p2(b0)).
        msk = const.tile([P, P], BF16)
        wk_sb = res.tile([P, HPC * NRC * DN], BF16)
        wv_sb = res.tile([P, NRC * HPC * DV], BF16)
        kv_sb = []
        for blk in range(NBLK):
            t = res.tile([P, NRC * BCOLS], BF16, tag=f"kv{blk}", name=f"kv{blk}")
            kv_sb.append(t)
        qn_sb = []
        for h in range(HPC):
            t = res.tile([DN, N], BF16, tag=f"qn{h}", name=f"qn{h}")
            qn_sb.append(t)
        # rope q/k on 128 partitions with rows 64-127 ZEROED: rope matmuls
        # run at contract 128 (zero rows contribute nothing), so the PE
        # stays in 128x128 tiling mode for the whole kernel -- no 64-mode
        # switch drains.  Both sides are zeroed so 0*0 can't make NaN.
        qr_sb = res.tile([P, HPC * N], BF16)
        kr_sb = res.tile([P, N], BF16)
        nc.gpsimd.memset(qr_sb[DR:P, :], 0.0)
        nc.gpsimd.memset(kr_sb[DR:P, :], 0.0)

        # Per-transfer completion latency is ~1-3us regardless of size (16-way
        # SDMA striping + completion descriptor) and each ring moves only
        # ~150 GB/s, so transfers must stay COARSE and CONTIGUOUS (a strided
        # kv0 experiment ran at 40 GB/s and sank the whole head).  Each ring
        # recycles 4 completion sems, so the N+4th issue instruction BLOCKS
        # its engine queue until transfer N completes: only 4 issues go in
        # up front; everything else is injected between p1 blocks (below) so
        # waiting issues never sit ahead of ACT/DVE compute in a queue.
        HKV = NRC * BCOLS // 2
        nc.scalar.dma_start(kv_sb[0][:, 0:HKV], kvt[0, :, 0:HKV])
        nc.sync.dma_start(wv_sb[:], wvt[:])
        nc.scalar.dma_start(kv_sb[0][:, HKV:], kvt[0, :, HKV:])
        nc.sync.dma_start(wk_sb[:], wkt[:])
        nc.scalar.dma_start(kv_sb[2][:], kvt[2])
        nc.sync.dma_start(kv_sb[1][:], kvt[1])
        nc.scalar.dma_start(kv_sb[4][:], kvt[4])
        nc.sync.dma_start(kv_sb[3][:], kvt[3])

        H1 = HPC * N - N  # qr col base of head 1
        late_dmas = {
            0: [(nc.sync, kv_sb[5][:], kvt[5]),
                (nc.sync, qn_sb[0][:, 0:S], qtn[0, :, 0:S]),
                (nc.scalar, qn_sb[1][:, 0:S], qtn[1, :, 0:S])],
            1: [(nc.sync, kr_sb[0:DR, 0:S], krt[:, 0:S]),
                (nc.sync, qr_sb[0:DR, 0:S], qrt[:, 0:S]),
                (nc.scalar, qr_sb[0:DR, H1:H1 + S], qrt[:, H1:H1 + S]),
                (nc.scalar, msk[:], mskd[:])],
            2: [(nc.scalar, kv_sb[6][:], kvt[6]),
                (nc.scalar, kv_sb[7][:], kvt[7])],
            3: [(nc.sync, kr_sb[0:DR, S:], krt[:, S:]),
                (nc.sync, qr_sb[0:DR, S:N], qrt[:, S:N]),
                (nc.scalar, qn_sb[0][:, S:], qtn[0, :, S:])],
            4: [(nc.sync, qr_sb[0:DR, N + S:], qrt[:, N + S:]),
                (nc.scalar, qn_sb[1][:, S:], qtn[1, :, S:])],
        }

        def kv(c, blk, lo, hi):  # cols [lo,hi) of r-chunk c within block blk
            return kv_sb[blk][:, c * BCOLS + lo:c * BCOLS + hi]

        kn_sb = [
            res.tile([P, N], BF16, tag=f"kn{h}", name=f"kn{h}") for h in range(HPC)
        ]
        # combined v_aug for both heads; chunk ki at [:, ki*VCH : (ki+1)*VCH]
        # = [v_h0(128) | 1 | pad | v_h1(128) | 1 | pad]
        vcomb = res.tile([P, (N // P) * VCH], BF16)
        vch_view = vcomb[:].rearrange("p (k v) -> p k v", v=VCH)
        nc.gpsimd.memset(vch_view[:, :, DV:DVAP], 1.0)
        nc.gpsimd.memset(vch_view[:, :, DVAP + DV:2 * DVAP], 1.0)

        # ---- Phase 1 for one 512-column block ----
        def p1_block(blk, flush_mid=False, copy_act=False):
            # v for both heads, two token-chunks packed per 1-bank PSUM tile
            # (512 f32 exactly): halves psO pool pressure and merges the two
            # evacuation copies into one.  Back-to-back same-region PSUM
            # accumulates are safe (the next matmul's ~128-cycle fill exceeds
            # the previous one's drain).  (blk0's first matmuls only need the
            # first kv0 half: the c-loop is outermost and chunks c=0,1 ship
            # as the first transfer.)  flush_mid drains one pending PV pair
            # before each v-half, hiding the p2->p1 exp straggler under v
            # matmuls instead of a PE bubble.
            HD = HPC * DV
            for half in (0, 1):
                if flush_mid and pend:
                    flush_one()
                psv = psO.tile([P, 2 * HD], F32, tag="psO",
                               name=f"vb{blk}_{half}")
                for c in range(NRC):
                    for kk in range(2):
                        ki = 2 * half + kk
                        # start=True clears has_written for the WHOLE bank, so
                        # only the bank's first matmul (c=0, kk=0) may use it;
                        # kk=1's first write overwrites via the cleared bits
                        nc.tensor.matmul(
                            psv[:, kk * HD:(kk + 1) * HD],
                            lhsT=kv(c, blk, ki * P, (ki + 1) * P),
                            rhs=wv_sb[:, c * HD:(c + 1) * HD],
                            start=(c == 0 and kk == 0), stop=(c == NRC - 1),
                            skip_group_check=True,
                        )
                kg = blk * (BCOLS // P) + 2 * half
                dst = vcomb[:, kg * VCH:(kg + 2) * VCH]
                # copy_act would move this to ACT -- measured neutral (the
                # boundary PE stalls are exp-latency echoes, not DVE queue
                # congestion), so everything stays on DVE
                ce = nc.scalar.copy if copy_act else nc.vector.tensor_copy
                ce(
                    dst.rearrange("p (k h d) -> p k h d", k=2,
                                  h=HPC)[:, :, :, 0:DV],
                    psv[:].rearrange("p (k h d) -> p k h d", k=2, h=HPC),
                )
            # k_nopeT both heads in one 2-bank tile, h-interleaved
            ps = psA.tile([P, 2 * QBLK], F32, tag="psA", name=f"knb{blk}")
            for c in range(NRC):
                for h in range(HPC):
                    nc.tensor.matmul(
                        ps[:, h * QBLK:h * QBLK + BCOLS],
                        lhsT=wk_sb[:, (h * NRC + c) * DN:(h * NRC + c + 1) * DN],
                        rhs=kv(c, blk, 0, BCOLS),
                        start=(c == 0), stop=(c == NRC - 1),
                        skip_group_check=True,
                    )
            js = slice(blk * BCOLS, (blk + 1) * BCOLS)
            for h in range(HPC):
                # blocks emitted mid-phase-2 copy on DVE: the ACT queue is 20+
                # exps deep there and would stall the next batch's scores
                # early blocks' kn copies ride DVE: the scalar queue carries
                # DMA issue instructions whose semaphore gates would stall
                # queued copies (and with them the psA ring) for us at a time
                (nc.scalar.copy if copy_act else nc.vector.tensor_copy)(
                    kn_sb[h][:, js], ps[:, h * QBLK:h * QBLK + BCOLS])

        # ---- Phase 2: flat pair stream over (head, batch, q-block) with the
        # PV software pipeline spanning block boundaries, so the PE never
        # drains at a q-block or head transition.
        class QBlkState:
            def __init__(self, h, b, qb):
                self.h, self.q0, self.qs = h, b * S, qb * QBLK
                self.nfull = self.qs // P
                self.kis = self.nfull + QBLK // P
                self.ops = [
                    psO.tile([P, 2 * DVAP], F32, tag="psO", name=f"opair{p_}")
                    for p_ in range(2)
                ]
                self.oq = osb.tile([P, 4 * DVAO], BF16, tag="oq", name="oq")
                self.early = False
                self.last = False  # kernel's very last q-block

        def emit_pv(qx, ki, j, qoff, pr):
            kidx = (qx.q0 + ki * P) // P
            h = qx.h
            va = vcomb[:, kidx * VCH + h * DVAP:kidx * VCH + h * DVAP + DVA]
            for j4 in range(max(0, j), QBLK // P):
                # start=True clears has_written for the WHOLE bank, so only the
                # first write of each bank-packed pair may use it; the
                # partner's first matmul overwrites via the cleared bits.
                nc.tensor.matmul(
                    qx.ops[j4 // 2][:, (j4 % 2) * DVAP:(j4 % 2) * DVAP + DVA],
                    lhsT=pr[:, j4 * P - qoff:(j4 + 1) * P - qoff],
                    rhs=va,
                    start=(ki == 0 and j4 % 2 == 0),
                    stop=(ki == qx.nfull + j4),
                    skip_group_check=True,
                )

        def epi2(qx, pair):
            # unnormalized o + denominator columns for BOTH subtiles of the
            # pair in one cast (DVAP == DVAO, so the 260-col copy lands with
            # the right per-subtile stride; the pad col carries PSUM garbage
            # the host never reads)
            nc.vector.tensor_copy(
                qx.oq[:, pair * 2 * DVAO:(pair + 1) * 2 * DVAO],
                qx.ops[pair][:],
            )

        def dma_half(qx, pair):
            # output DMAs ride the sync HWDGE ring, which is idle once the
            # input issues finish: no ACT issue cost, and HWDGE completion
            # drains ~5us faster at kernel end than GPSIMD SWDGE.  Each
            # q-block ships as two half-DMAs so the first half leaves while
            # the last pair is still in PV (shrinks the kernel-end drain).
            qg = (qx.q0 + qx.qs) // QBLK
            nc.sync.dma_start(
                out[qx.h, qg, :, pair * 2 * DVAO:(pair + 1) * 2 * DVAO],
                qx.oq[:, pair * 2 * DVAO:(pair + 1) * 2 * DVAO],
            )

        def finalize(qx):
            epi2(qx, 1)
            if qx.last:
                # the kernel's final transfer is the last exec-counted event:
                # split it by PARTITION halves across both rings (keeps the
                # 520B/partition packets) so transfer + completion latency
                # run in parallel on two engines
                qg = (qx.q0 + qx.qs) // QBLK
                nc.sync.dma_start(
                    out[qx.h, qg, 0:64, 2 * DVAO:4 * DVAO],
                    qx.oq[0:64, 2 * DVAO:4 * DVAO],
                )
                nc.scalar.dma_start(
                    out[qx.h, qg, 64:P, 2 * DVAO:4 * DVAO],
                    qx.oq[64:P, 2 * DVAO:4 * DVAO],
                )
            else:
                dma_half(qx, 1)

        # FIFO of up to two pending pairs: PV for pair n-2 streams while ACT
        # computes exp(n-1) and exp(n), so a block-boundary exp straggler
        # never stalls the PE.
        pend = []

        def flush_one():
            qx, plist = pend.pop(0)
            for (ki, j, qoff, prsl) in plist:
                emit_pv(qx, ki, j, qoff, prsl)
                if ki == qx.nfull + 1 and not qx.early:
                    # pair 0 (subtiles 0,1) complete: copy + DMA out now so
                    # its PSUM bank frees early and the output is half-shipped
                    # before the q-block's last PV finishes
                    epi2(qx, 0)
                    dma_half(qx, 0)
                    qx.early = True
            if plist[-1][0] == qx.kis - 1:
                finalize(qx)

        def p2_run(hb_list, final=False, drain=True):
            for (h, b) in hb_list:
                for qb in range(S // QBLK):
                    qx = QBlkState(h, b, qb)
                    qx.last = (final and (h, b) == hb_list[-1]
                               and qb == S // QBLK - 1)
                    for k1 in range(0, qx.kis, 2):
                        metas = []
                        for ki, base in ((k1, 0), (k1 + 1, QBLK)):
                            if ki < qx.nfull:
                                metas.append((ki, -1, 0, QBLK, base))
                            else:
                                j = ki - qx.nfull
                                metas.append((ki, j, j * P, QBLK - j * P, base))
                        # diag pairs that fit one PSUM bank (256+128) pack
                        # side by side: the pair's exp span drops 640 -> 384
                        # cols, shortening the straggler every q-block-end
                        # flush waits on
                        if metas[0][3] + metas[1][3] <= QBLK:
                            ki1, j1, qoff1, w1b = metas[1][:4]
                            metas[1] = (ki1, j1, qoff1, w1b, metas[0][3])
                        sc2 = psA.tile([P, 2 * QBLK], F32, tag="psA", name="sc2")
                        # scores interleaved n1,n2,r1,r2: same-bank accumulate
                        # pairs (n_i -> r_i) separated by another matmul.
                        # Rope is ZERO-PADDED to contract 128 (kr/qr rows
                        # 64-127 memset to 0): ~90ns slower streaming per pair
                        # than 64-row tiling, but the PE never leaves 128x128
                        # tiling mode, avoiding ~195ns of mode-switch drains
                        # per pair (measured net win).
                        for mi, (ki, j, qoff, w, base) in enumerate(metas):
                            kg = qx.q0 + ki * P
                            qg = qx.q0 + qx.qs + qoff
                            # a packed partner (base < QBLK, same bank) must
                            # NOT use start=True: it would clear has_written
                            # for the whole bank and wipe meta0's scores
                            nc.tensor.matmul(
                                sc2[:, base:base + w],
                                lhsT=kn_sb[h][:, kg:kg + P],
                                rhs=qn_sb[h][:, qg:qg + w],
                                start=(mi == 0 or base >= QBLK), stop=False,
                                skip_group_check=True,
                            )
                        for (ki, j, qoff, w, base) in metas:
                            kg = qx.q0 + ki * P
                            qg = qx.q0 + qx.qs + qoff
                            nc.tensor.matmul(
                                sc2[:, base:base + w],
                                lhsT=kr_sb[:, kg:kg + P],
                                rhs=qr_sb[:, h * N + qg:h * N + qg + w],
                                start=False, stop=True, skip_group_check=True,
                            )
                        pr2 = prs.tile([P, 2 * QBLK], BF16, tag="probs",
                                       name="pr2")
                        # one exp covers the pair (cols [w1, 512) may hold
                        # garbage; nothing reads them) -- except the kernel's
                        # very last pair, whose exp is fully exposed on the
                        # critical tail: per-chunk exps let its PV start half
                        # an exp earlier
                        span = metas[1][4] + metas[1][3]
                        if (final and (h, b) == hb_list[-1]
                                and qb == S // QBLK - 1 and k1 + 2 >= qx.kis):
                            nc.scalar.activation(pr2[:, :metas[0][3]],
                                                 sc2[:, :metas[0][3]], Exp,
                                                 scale=SCALE, bias=ebias[:])
                            nc.scalar.activation(pr2[:, metas[1][4]:span],
                                                 sc2[:, metas[1][4]:span],
                                                 Exp, scale=SCALE,
                                                 bias=ebias[:])
                        else:
                            nc.scalar.activation(pr2[:, :span], sc2[:, :span],
                                                 Exp, scale=SCALE,
                                                 bias=ebias[:])
                        # multiplicative causal mask on the diagonal block
                        for (ki, j, qoff, w, base) in metas:
                            if j >= 0:
                                nc.vector.tensor_mul(
                                    pr2[:, base:base + P],
                                    pr2[:, base:base + P], msk[:],
                                )
                        # software pipeline: PV trails two pairs behind
                        # (possibly crossing q-block/head boundaries)
                        if len(pend) >= 2:
                            flush_one()
                        pend.append((qx, [
                            (ki, j, qoff, pr2[:, base:base + w])
                            for (ki, j, qoff, w, base) in metas
                        ]))
            # drain before leaving p2 (p1 blocks reuse the PSUM rings) --
            # unless the following p1 block interleaves the flush itself
            if drain:
                while pend:
                    flush_one()

        # ---- emission schedule: front-load P1 past the DMA ramp, then
        # interleave so q/k data for later batches arrives under compute.
        for blk in range(6):
            p1_block(blk)
            for (eng, dst, src) in late_dmas.get(blk, []):
                eng.dma_start(dst, src)
        p2_run([(h, 0) for h in range(HPC)], drain=False)
        p1_block(6, flush_mid=True)
        p1_block(7, flush_mid=True)
        p2_run([(h, b) for b in range(1, B) for h in range(HPC)], final=True)

    nc.compile()
    return nc


def _prep_inputs(q, k, w_key, w_vo):
    bf = ml_dtypes.bfloat16
    kv_c = np.ascontiguousarray(k[:, 0, :R])          # [N, 512]
    k_rope = np.ascontiguousarray(k[:, 0, R:])        # [N, 64]
    # kvt[blk][rl, c*BCOLS+nl] = kv_c[blk*BCOLS+nl, c*128+rl]
    kvt = np.ascontiguousarray(
        kv_c.T.reshape(NRC, P, NBLK, BCOLS).transpose(2, 1, 0, 3)
        .reshape(NBLK, P, NRC * BCOLS).astype(bf))
    krt = np.ascontiguousarray(k_rope.T.astype(bf))   # [64, N]
    msk = np.triu(np.ones((P, P), np.float32)).astype(bf)  # 1 where k <= q

    in_maps = []
    for core in range(NCORES):
        hs = slice(core * HPC, (core + 1) * HPC)
        qh = q[:, hs, :]                              # [N, HPC, 192]
        qtn = np.ascontiguousarray(
            qh[:, :, :DN].transpose(1, 2, 0).astype(bf))   # [HPC, 128, N]
        # qrt[rl, h*N + n] = q[n, h, 128+rl]
        qrt = np.ascontiguousarray(
            qh[:, :, DN:].transpose(2, 1, 0).reshape(DR, HPC * N).astype(bf))
        # wkt[rl, (h*NRC+c)*DN + d] = w_key[hs][h, d, c*128+rl]
        wkt = np.ascontiguousarray(
            w_key[hs].transpose(2, 0, 1)              # [512r, HPC, 128d]
            .reshape(NRC, P, HPC, DN).transpose(1, 2, 0, 3)
            .reshape(P, HPC * NRC * DN).astype(bf))
        # wvt[rl, (c*HPC+h)*DV + d] = w_vo[hs][h, d, c*128+rl]
        wvt = np.ascontiguousarray(
            w_vo[hs].transpose(2, 0, 1)               # [512r, HPC, 128d]
            .reshape(NRC, P, HPC * DV).transpose(1, 0, 2)
            .reshape(P, NRC * HPC * DV).astype(bf))
        in_maps.append({
            "qtn": qtn, "qrt": qrt, "kvt": kvt, "krt": krt,
            "wkt": wkt, "wvt": wvt, "mskd": msk,
        })
    return in_maps


def run(q, k, v, w_key, w_vo, trace=False, tmpdir=None):
    """Returns (output [N, H, 128] f32, BassKernelResults)."""
    if "nc" not in _CACHE:
        _CACHE["nc"] = _build()
    nc = _CACHE["nc"]
    in_maps = _prep_inputs(np.asarray(q), np.asarray(k),
                           np.asarray(w_key), np.asarray(w_vo))
    res = run_bass_kernel_spmd(
        nc, in_maps, core_ids=list(range(NCORES)), trace=trace, tmpdir=tmpdir
    )
    outs = [np.asarray(res.results[i]["out"]).astype(np.float32)
            for i in range(NCORES)]
    full = np.concatenate(outs, axis=0)                # [16, 8, 128, 520]
    # device layout [h, qblock, p, j*130 + d] -> token (qblock*512 + j*128 + p)
    full = (full.reshape(H, N // QBLK, P, 4, DVAO).transpose(0, 1, 3, 2, 4)
            .reshape(H, N, DVAO))
    o = full[:, :, :DV] / full[:, :, DV:DV + 1]        # host-side softmax denom
    return np.ascontiguousarray(o.transpose(1, 0, 2)), res


def kernel(q, k, v, w_key, w_vo):
    return run(q, k, v, w_key, w_vo)[0]



# revision 40
# speedup vs baseline: 1.0034x; 1.0034x over previous
"""MLA prefill attention (DeepSeek-style), tensor-parallel over heads on 8 TRN2 NeuronCores.

Reference computation (per head h, per batch b of 4 x 1024 tokens):
  kv_c   = k[:, 0, :512]                  # [N, 512] compressed latent (shared)
  k_nope = kv_c @ w_key[h].T              # [N, 128]
  k_full = concat(k_nope, k_rope)         # [N, 192]
  v_raw  = kv_c @ w_vo[h].T               # [N, 128]
  o      = softmax(causal(q_h @ k_full.T * SCALE)) @ v_raw

Sharding: 16 heads / 8 cores = 2 heads per core; kv_c replicated. No collectives.

Device kernel (per core, all matmuls bf16):
  Phase 1 per 512-token block: v for both heads, two 128-token chunks packed
    per 1-bank PSUM tile (512 f32 exactly; only the bank's first matmul may
    use start=True -- it clears has_written for the WHOLE bank), evacuated by
    ONE merged DVE copy per tile; k_nopeT for both heads into one 2-bank
    PSUM tile (h-interleaved).  Diagonal score pairs that fit one bank
    (256+128) also pack side by side, cutting their exp span 640 -> 384
    cols.  ALL PSUM evacuation copies ride DVE: the
    scalar queue carries DMA-issue instructions whose semaphore gates would
    stall queued copies (and the PSUM rings behind them) for microseconds.
  Phase 2: transposed-score flash attention processed in chunk PAIRS: the two
    chunks' score matmuls land in the two banks of one [128,1024] PSUM tile,
    interleaved (n1,n2,r1,r2) to hide accumulation drains, then ONE exp
    activation covers the pair (halves ACT instruction overhead). Causal
    triangle masked multiplicatively on probs (bf16, SBUF) after exp, off the
    PSUM path. PV uses probs blocks as stationary with v_aug moving
    ([v|1] -> softmax denominator rides in column 128). Output is written
    UNNORMALIZED as bf16 [o(128) | den | pad] per token; the host divides.
  The PV software pipeline trails TWO pairs behind the scores and spans
    q-block/head boundaries, so a boundary exp straggler never drains the PE.
    At the p2(b0) -> p1(6) re-entry the two pending flushes interleave with
    p1's v-halves (flush_mid), hiding the final exp stragglers under v
    matmuls instead of a ~1us PE bubble.
  Rope matmuls are ZERO-PADDED to contract 128 (kr/qr SBUF rows 64-127
    memset to 0): the PE never leaves 128x128 tiling mode.  The 64-contract
    alternative pays ~195ns of tiling-mode-switch drains per pair (measured),
    which exceeds the extra streaming cost of the zero rows.
  PE WARM-UP: ~10 throwaway matmuls from a memset scratch tile run during
    the engine-boot + DMA ramp dead time (first ~10us), so the HAM clock
    gate is at 2.4 GHz when real data lands (throttle time 11.5us -> 7us).
  Emission order p1(blk 0..5), p2(batch0), p1(6), p1(7), p2(b1..b3) keeps the
  PE busy through the input-DMA ramp (q / later kv blocks arrive while early
  blocks compute). Inputs split across both HWDGE rings (sync + scalar) in
  arrival-priority order; each ring recycles only 4 completion semaphores,
  so the 5th+ issue WOULD block its engine queue (and with it ACT's kn
  copies) -- later dma_starts are therefore injected BETWEEN p1 block
  emissions.  All transfers are contiguous (a strided kv0 experiment ran at
  ~40 GB/s vs ~150 per ring).  Outputs ride the sync ring p-major ([h, qblk,
  p, 4*130], 1040B/partition rows, 4x bigger packets than token-major) in
  two half-DMAs per q-block, the first issued mid-q-block so the kernel-end
  drain is one 66KB transfer; the host unscrambles + divides.

Measured on 8-core axon TRN2: 98.3us (session start: 110.9us; first
session's baseline: 141us), rel fro err 4.7e-3.  exec_time is run-variable
(+-3us; occasional whole runs at 2.0 GHz P0 power state) -- compare kernels
only within one process (test_ab.py).
"""

import os
import sys

sys.path.insert(0, "/opt/trn_rl_repo")

from contextlib import ExitStack

import numpy as np
import ml_dtypes

import concourse.bass as bass
import concourse.mybir as mybir
from concourse import bacc, tile
from concourse.bass_utils import run_bass_kernel_spmd

B, S, H, N = 4, 1024, 16, 4096
DN, DR, DV, R = 128, 64, 128, 512
SCALE = 0.07216878364870323
NCORES = 8
HPC = H // NCORES  # heads per core
P = 128
QBLK = 512
NRC = R // P  # 4 r-chunks
NBLK = 8      # kv column blocks
BCOLS = N // NBLK
DVA = DV + 1    # v | ones  -> rowsums fall out of PV
DVAP = DV + 2   # pair stride padded so both PSUM slices are 8B-aligned
DVAO = DV + 2   # output stride per token: [o(128) | den | pad]
VCH = 2 * DVAP  # both-heads v chunk stride [v0 | 1 | pad | v1 | 1 | pad]
BF16 = mybir.dt.bfloat16
F32 = mybir.dt.float32
Exp = mybir.ActivationFunctionType.Exp
EXP_BIAS = -2.5  # shift-invariant softmax bias keeps exp outputs small

_CACHE: dict = {}


def _build():
    nc = bacc.Bacc("TRN2", target_bir_lowering=False, debug=False, num_devices=NCORES)

    qtn = nc.dram_tensor("qtn", [HPC, DN, N], BF16, kind="ExternalInput").ap()
    # rope q packed both heads on 64 partitions: [64, h*N + n]
    qrt = nc.dram_tensor("qrt", [DR, HPC * N], BF16, kind="ExternalInput").ap()
    # per column-block, r-chunks side by side: [blk][128r, c*BCOLS + n]
    kvt = nc.dram_tensor("kvt", [NBLK, P, NRC * BCOLS], BF16,
                         kind="ExternalInput").ap()
    krt = nc.dram_tensor("krt", [DR, N], BF16, kind="ExternalInput").ap()
    # w_key both heads: [128r(of chunk c), (h*NRC + c)*DN + d]
    wkt = nc.dram_tensor("wkt", [P, HPC * NRC * DN], BF16, kind="ExternalInput").ap()
    # w_vo both heads per r-chunk: [128r, (c*HPC + h)*DV + d]
    wvt = nc.dram_tensor("wvt", [P, NRC * HPC * DV], BF16, kind="ExternalInput").ap()
    mskd = nc.dram_tensor("mskd", [P, P], BF16, kind="ExternalInput").ap()
    # unnormalized output + denominator, p-major per q-block so each SBUF
    # partition DMAs one contiguous 1040B row (4x bigger packets than
    # token-major); host divides + unscrambles
    out = nc.dram_tensor("out", [HPC, N // QBLK, P, 4 * DVAO], BF16,
                         kind="ExternalOutput").ap()

    with tile.TileContext(nc) as tc, ExitStack() as ctx:
        const = ctx.enter_context(tc.tile_pool(name="const", bufs=1))
        res = ctx.enter_context(tc.tile_pool(name="res", bufs=1))
        prs = ctx.enter_context(tc.tile_pool(name="prs", bufs=4))
        osb = ctx.enter_context(tc.tile_pool(name="osb", bufs=4))
        psA = ctx.enter_context(tc.tile_pool(name="psA", bufs=2, space="PSUM"))
        psO = ctx.enter_context(tc.tile_pool(name="psO", bufs=4, space="PSUM"))

        ebias = const.tile([P, 1], F32)
        nc.gpsimd.memset(ebias[:], EXP_BIAS)

        # ---- PE warm-up: the HAM clock gate keeps the PE at 1.2 GHz until
        # ~3.4us of sustained matmul activity.  The first real matmul can't
        # start until ~10.5us (engine boot + DMA ramp), so burn that dead time
        # on throwaway matmuls from a memset scratch tile: the PE is at
        # 2.4 GHz the moment real data lands (saves the ~4.8us cold penalty).
        scr_w = const.tile([P, QBLK], BF16)
        nc.gpsimd.memset(scr_w[:], 0.03)
        scr_ps = psA.tile([P, 2 * QBLK], F32, tag="psA", name="warm")
        for wi in range(10):
            nc.tensor.matmul(
                scr_ps[:, (wi % 2) * QBLK:(wi % 2) * QBLK + QBLK],
                lhsT=scr_w[:, 0:P], rhs=scr_w[:],
                start=True, stop=True, skip_group_check=True,
            )

        # ---- input DMAs: need-sorted, alternating between the two HWDGE
        # rings (sync + scalar) so the rings' packet-granular round-robin over
        # the 16 SDMA engines reproduces a single priority-ordered stream at
        # full bandwidth.  Need order: kv0a/wv (first v group), kv0b/wk
        # (second group + k_nope), kv1..kv5, then q/kr for p2(b0) split so the
        # batch-0 slices land first, then kv6/kv7 (p1 resumes after p2(b0)).
        msk = const.tile([P, P], BF16)
        wk_sb = res.tile([P, HPC * NRC * DN], BF16)
        wv_sb = res.tile([P, NRC * HPC * DV], BF16)
        kv_sb = []
        for blk in range(NBLK):
            t = res.tile([P, NRC * BCOLS], BF16, tag=f"kv{blk}", name=f"kv{blk}")
            kv_sb.append(t)
        qn_sb = []
        for h in range(HPC):
            t = res.tile([DN, N], BF16, tag=f"qn{h}", name=f"qn{h}")
            qn_sb.append(t)
        # rope q/k on 128 partitions with rows 64-127 ZEROED: rope matmuls
        # run at contract 128 (zero rows contribute nothing), so the PE
        # stays in 128x128 tiling mode for the whole kernel -- no 64-mode
        # switch drains.  Both sides are zeroed so 0*0 can't make NaN.
        qr_sb = res.tile([P, HPC * N], BF16)
        kr_sb = res.tile([P, N], BF16)
        nc.gpsimd.memset(qr_sb[DR:P, :], 0.0)
        nc.gpsimd.memset(kr_sb[DR:P, :], 0.0)

        # Per-transfer completion latency is ~1-3us regardless of size (16-way
        # SDMA striping + completion descriptor) and each ring moves only
        # ~150 GB/s, so transfers must stay COARSE and CONTIGUOUS (a strided
        # kv0 experiment ran at 40 GB/s and sank the whole head).  Each ring
        # recycles 4 completion sems, so the N+4th issue instruction BLOCKS
        # its engine queue until transfer N completes: only 4 issues go in
        # up front; everything else is injected between p1 blocks (below) so
        # waiting issues never sit ahead of ACT/DVE compute in a queue.
        HKV = NRC * BCOLS // 2
        nc.scalar.dma_start(kv_sb[0][:, 0:HKV], kvt[0, :, 0:HKV])
        nc.sync.dma_start(wv_sb[:], wvt[:])
        nc.scalar.dma_start(kv_sb[0][:, HKV:], kvt[0, :, HKV:])
        nc.sync.dma_start(wk_sb[:], wkt[:])
        nc.scalar.dma_start(kv_sb[2][:], kvt[2])
        nc.sync.dma_start(kv_sb[1][:], kvt[1])
        nc.scalar.dma_start(kv_sb[4][:], kvt[4])
        nc.sync.dma_start(kv_sb[3][:], kvt[3])

        H1 = HPC * N - N  # qr col base of head 1
        late_dmas = {
            0: [(nc.sync, kv_sb[5][:], kvt[5]),
                (nc.sync, qn_sb[0][:, 0:S], qtn[0, :, 0:S]),
                (nc.scalar, qn_sb[1][:, 0:S], qtn[1, :, 0:S])],
            1: [(nc.sync, kr_sb[0:DR, 0:S], krt[:, 0:S]),
                (nc.sync, qr_sb[0:DR, 0:S], qrt[:, 0:S]),
                (nc.scalar, qr_sb[0:DR, H1:H1 + S], qrt[:, H1:H1 + S]),
                (nc.scalar, msk[:], mskd[:])],
            2: [(nc.scalar, kv_sb[6][:], kvt[6]),
                (nc.scalar, kv_sb[7][:], kvt[7])],
            3: [(nc.sync, kr_sb[0:DR, S:], krt[:, S:]),
                (nc.sync, qr_sb[0:DR, S:N], qrt[:, S:N]),
                (nc.scalar, qn_sb[0][:, S:], qtn[0, :, S:])],
            4: [(nc.sync, qr_sb[0:DR, N + S:], qrt[:, N + S:]),
                (nc.scalar, qn_sb[1][:, S:], qtn[1, :, S:])],
        }

        def kv(c, blk, lo, hi):  # cols [lo,hi) of r-chunk c within block blk
            return kv_sb[blk][:, c * BCOLS + lo:c * BCOLS + hi]

        kn_sb = [
            res.tile([P, N], BF16, tag=f"kn{h}", name=f"kn{h}") for h in range(HPC)
        ]
        # combined v_aug for both heads; chunk ki at [:, ki*VCH : (ki+1)*VCH]
        # = [v_h0(128) | 1 | pad | v_h1(128) | 1 | pad]
        vcomb = res.tile([P, (N // P) * VCH], BF16)
        vch_view = vcomb[:].rearrange("p (k v) -> p k v", v=VCH)
        nc.gpsimd.memset(vch_view[:, :, DV:DVAP], 1.0)
        nc.gpsimd.memset(vch_view[:, :, DVAP + DV:2 * DVAP], 1.0)

        # ---- Phase 1 for one 512-column block ----
        def p1_block(blk, flush_mid=False, copy_act=False):
            # v for both heads, two token-chunks packed per 1-bank PSUM tile
            # (512 f32 exactly): halves psO pool pressure and merges the two
            # evacuation copies into one.  Back-to-back same-region PSUM
            # accumulates are safe (the next matmul's ~128-cycle fill exceeds
            # the previous one's drain).  (blk0's first matmuls only need the
            # first kv0 half: the c-loop is outermost and chunks c=0,1 ship
            # as the first transfer.)  flush_mid drains one pending PV pair
            # before each v-half, hiding the p2->p1 exp straggler under v
            # matmuls instead of a PE bubble.
            HD = HPC * DV
            for half in (0, 1):
                if flush_mid and pend:
                    flush_one()
                psv = psO.tile([P, 2 * HD], F32, tag="psO",
                               name=f"vb{blk}_{half}")
                for c in range(NRC):
                    for kk in range(2):
                        ki = 2 * half + kk
                        # start=True clears has_written for the WHOLE bank, so
                        # only the bank's first matmul (c=0, kk=0) may use it;
                        # kk=1's first write overwrites via the cleared bits
                        nc.tensor.matmul(
                            psv[:, kk * HD:(kk + 1) * HD],
                            lhsT=kv(c, blk, ki * P, (ki + 1) * P),
                            rhs=wv_sb[:, c * HD:(c + 1) * HD],
                            start=(c == 0 and kk == 0), stop=(c == NRC - 1),
                            skip_group_check=True,
                        )
                kg = blk * (BCOLS // P) + 2 * half
                dst = vcomb[:, kg * VCH:(kg + 2) * VCH]
                # copy_act would move this to ACT -- measured neutral (the
                # boundary PE stalls are exp-latency echoes, not DVE queue
                # congestion), so everything stays on DVE
                ce = nc.scalar.copy if copy_act else nc.vector.tensor_copy
                ce(
                    dst.rearrange("p (k h d) -> p k h d", k=2,
                                  h=HPC)[:, :, :, 0:DV],
                    psv[:].rearrange("p (k h d) -> p k h d", k=2, h=HPC),
                )
            # k_nopeT both heads in one 2-bank tile, h-interleaved
            ps = psA.tile([P, 2 * QBLK], F32, tag="psA", name=f"knb{blk}")
            for c in range(NRC):
                for h in range(HPC):
                    nc.tensor.matmul(
                        ps[:, h * QBLK:h * QBLK + BCOLS],
                        lhsT=wk_sb[:, (h * NRC + c) * DN:(h * NRC + c + 1) * DN],
                        rhs=kv(c, blk, 0, BCOLS),
                        start=(c == 0), stop=(c == NRC - 1),
                        skip_group_check=True,
                    )
            js = slice(blk * BCOLS, (blk + 1) * BCOLS)
            for h in range(HPC):
                # blocks emitted mid-phase-2 copy on DVE: the ACT queue is 20+
                # exps deep there and would stall the next batch's scores
                # early blocks' kn copies ride DVE: the scalar queue carries
                # DMA issue instructions whose semaphore gates would stall
                # queued copies (and with them the psA ring) for us at a time
                (nc.scalar.copy if copy_act else nc.vector.tensor_copy)(
                    kn_sb[h][:, js], ps[:, h * QBLK:h * QBLK + BCOLS])

        # ---- Phase 2: flat pair stream over (head, batch, q-block) with the
        # PV software pipeline spanning block boundaries, so the PE never
        # drains at a q-block or head transition.
        class QBlkState:
            def __init__(self, h, b, qb):
                self.h, self.q0, self.qs = h, b * S, qb * QBLK
                self.nfull = self.qs // P
                self.kis = self.nfull + QBLK // P
                self.ops = [
                    psO.tile([P, 2 * DVAP], F32, tag="psO", name=f"opair{p_}")
                    for p_ in range(2)
                ]
                self.oq = osb.tile([P, 4 * DVAO], BF16, tag="oq", name="oq")
                self.early = False

        def emit_pv(qx, ki, j, qoff, pr):
            kidx = (qx.q0 + ki * P) // P
            h = qx.h
            va = vcomb[:, kidx * VCH + h * DVAP:kidx * VCH + h * DVAP + DVA]
            for j4 in range(max(0, j), QBLK // P):
                # start=True clears has_written for the WHOLE bank, so only the
                # first write of each bank-packed pair may use it; the
                # partner's first matmul overwrites via the cleared bits.
                nc.tensor.matmul(
                    qx.ops[j4 // 2][:, (j4 % 2) * DVAP:(j4 % 2) * DVAP + DVA],
                    lhsT=pr[:, j4 * P - qoff:(j4 + 1) * P - qoff],
                    rhs=va,
                    start=(ki == 0 and j4 % 2 == 0),
                    stop=(ki == qx.nfull + j4),
                    skip_group_check=True,
                )

        def epi2(qx, pair):
            # unnormalized o + denominator columns for BOTH subtiles of the
            # pair in one cast (DVAP == DVAO, so the 260-col copy lands with
            # the right per-subtile stride; the pad col carries PSUM garbage
            # the host never reads)
            nc.vector.tensor_copy(
                qx.oq[:, pair * 2 * DVAO:(pair + 1) * 2 * DVAO],
                qx.ops[pair][:],
            )

        def dma_half(qx, pair):
            # output DMAs ride the sync HWDGE ring, which is idle once the
            # input issues finish: no ACT issue cost, and HWDGE completion
            # drains ~5us faster at kernel end than GPSIMD SWDGE.  Each
            # q-block ships as two half-DMAs so the first half leaves while
            # the last pair is still in PV (shrinks the kernel-end drain).
            qg = (qx.q0 + qx.qs) // QBLK
            nc.sync.dma_start(
                out[qx.h, qg, :, pair * 2 * DVAO:(pair + 1) * 2 * DVAO],
                qx.oq[:, pair * 2 * DVAO:(pair + 1) * 2 * DVAO],
            )

        def finalize(qx):
            epi2(qx, 1)
            dma_half(qx, 1)

        # FIFO of up to two pending pairs: PV for pair n-2 streams while ACT
        # computes exp(n-1) and exp(n), so a block-boundary exp straggler
        # never stalls the PE.
        pend = []

        def flush_one():
            qx, plist = pend.pop(0)
            for (ki, j, qoff, prsl) in plist:
                emit_pv(qx, ki, j, qoff, prsl)
                if ki == qx.nfull + 1 and not qx.early:
                    # pair 0 (subtiles 0,1) complete: copy + DMA out now so
                    # its PSUM bank frees early and the output is half-shipped
                    # before the q-block's last PV finishes
                    epi2(qx, 0)
                    dma_half(qx, 0)
                    qx.early = True
            if plist[-1][0] == qx.kis - 1:
                finalize(qx)

        def p2_run(hb_list, final=False, drain=True):
            for (h, b) in hb_list:
                for qb in range(S // QBLK):
                    qx = QBlkState(h, b, qb)
                    for k1 in range(0, qx.kis, 2):
                        metas = []
                        for ki, base in ((k1, 0), (k1 + 1, QBLK)):
                            if ki < qx.nfull:
                                metas.append((ki, -1, 0, QBLK, base))
                            else:
                                j = ki - qx.nfull
                                metas.append((ki, j, j * P, QBLK - j * P, base))
                        # diag pairs that fit one PSUM bank (256+128) pack
                        # side by side: the pair's exp span drops 640 -> 384
                        # cols, shortening the straggler every q-block-end
                        # flush waits on
                        if metas[0][3] + metas[1][3] <= QBLK:
                            ki1, j1, qoff1, w1b = metas[1][:4]
                            metas[1] = (ki1, j1, qoff1, w1b, metas[0][3])
                        sc2 = psA.tile([P, 2 * QBLK], F32, tag="psA", name="sc2")
                        # scores interleaved n1,n2,r1,r2: same-bank accumulate
                        # pairs (n_i -> r_i) separated by another matmul.
                        # Rope is ZERO-PADDED to contract 128 (kr/qr rows
                        # 64-127 memset to 0): ~90ns slower streaming per pair
                        # than 64-row tiling, but the PE never leaves 128x128
                        # tiling mode, avoiding ~195ns of mode-switch drains
                        # per pair (measured net win).
                        for mi, (ki, j, qoff, w, base) in enumerate(metas):
                            kg = qx.q0 + ki * P
                            qg = qx.q0 + qx.qs + qoff
                            # a packed partner (base < QBLK, same bank) must
                            # NOT use start=True: it would clear has_written
                            # for the whole bank and wipe meta0's scores
                            nc.tensor.matmul(
                                sc2[:, base:base + w],
                                lhsT=kn_sb[h][:, kg:kg + P],
                                rhs=qn_sb[h][:, qg:qg + w],
                                start=(mi == 0 or base >= QBLK), stop=False,
                                skip_group_check=True,
                            )
                        for (ki, j, qoff, w, base) in metas:
                            kg = qx.q0 + ki * P
                            qg = qx.q0 + qx.qs + qoff
                            nc.tensor.matmul(
                                sc2[:, base:base + w],
                                lhsT=kr_sb[:, kg:kg + P],
                                rhs=qr_sb[:, h * N + qg:h * N + qg + w],
                                start=False, stop=True, skip_group_check=True,
                            )
                        pr2 = prs.tile([P, 2 * QBLK], BF16, tag="probs",
                                       name="pr2")
                        # one exp covers the pair (cols [w1, 512) may hold
                        # garbage; nothing reads them) -- except the kernel's
                        # very last pair, whose exp is fully exposed on the
                        # critical tail: per-chunk exps let its PV start half
                        # an exp earlier
                        span = metas[1][4] + metas[1][3]
                        if (final and (h, b) == hb_list[-1]
                                and qb == S // QBLK - 1 and k1 + 2 >= qx.kis):
                            nc.scalar.activation(pr2[:, :metas[0][3]],
                                                 sc2[:, :metas[0][3]], Exp,
                                                 scale=SCALE, bias=ebias[:])
                            nc.scalar.activation(pr2[:, metas[1][4]:span],
                                                 sc2[:, metas[1][4]:span],
                                                 Exp, scale=SCALE,
                                                 bias=ebias[:])
                        else:
                            nc.scalar.activation(pr2[:, :span], sc2[:, :span],
                                                 Exp, scale=SCALE,
                                                 bias=ebias[:])
                        # multiplicative causal mask on the diagonal block
                        for (ki, j, qoff, w, base) in metas:
                            if j >= 0:
                                nc.vector.tensor_mul(
                                    pr2[:, base:base + P],
                                    pr2[:, base:base + P], msk[:],
                                )
                        # software pipeline: PV trails two pairs behind
                        # (possibly crossing q-block/head boundaries)
                        if len(pend) >= 2:
                            flush_one()
                        pend.append((qx, [
                            (ki, j, qoff, pr2[:, base:base + w])
                            for (ki, j, qoff, w, base) in metas
                        ]))
            # drain before leaving p2 (p1 blocks reuse the PSUM rings) --
            # unless the following p1 block interleaves the flush itself
            if drain:
                while pend:
                    flush_one()

        # ---- emission schedule: front-load P1 past the DMA ramp, then
        # interleave so q/k data for later batches arrives under compute.
        for blk in range(6):
            p1_block(blk)
            for (eng, dst, src) in late_dmas.get(blk, []):
                eng.dma_start(dst, src)
        p2_run([(h, 0) for h in range(HPC)], drain=False)
        p1_block(6, flush_mid=True)
        p1_block(7, flush_mid=True)
        p2_run([(h, b) for b in range(1, B) for h in range(HPC)], final=True)

    nc.compile()
    return nc


def _prep_inputs(q, k, w_key, w_vo):
    bf = ml_dtypes.bfloat16
    kv_c = np.ascontiguousarray(k[:, 0, :R])          # [N, 512]
    k_rope = np.ascontiguousarray(k[:, 0, R:])        # [N, 64]
    # kvt[blk][rl, c*BCOLS+nl] = kv_c[blk*BCOLS+nl, c*128+rl]
    kvt = np.ascontiguousarray(
        kv_c.T.reshape(NRC, P, NBLK, BCOLS).transpose(2, 1, 0, 3)
        .reshape(NBLK, P, NRC * BCOLS).astype(bf))
    krt = np.ascontiguousarray(k_rope.T.astype(bf))   # [64, N]
    msk = np.triu(np.ones((P, P), np.float32)).astype(bf)  # 1 where k <= q

    in_maps = []
    for core in range(NCORES):
        hs = slice(core * HPC, (core + 1) * HPC)
        qh = q[:, hs, :]                              # [N, HPC, 192]
        qtn = np.ascontiguousarray(
            qh[:, :, :DN].transpose(1, 2, 0).astype(bf))   # [HPC, 128, N]
        # qrt[rl, h*N + n] = q[n, h, 128+rl]
        qrt = np.ascontiguousarray(
            qh[:, :, DN:].transpose(2, 1, 0).reshape(DR, HPC * N).astype(bf))
        # wkt[rl, (h*NRC+c)*DN + d] = w_key[hs][h, d, c*128+rl]
        wkt = np.ascontiguousarray(
            w_key[hs].transpose(2, 0, 1)              # [512r, HPC, 128d]
            .reshape(NRC, P, HPC, DN).transpose(1, 2, 0, 3)
            .reshape(P, HPC * NRC * DN).astype(bf))
        # wvt[rl, (c*HPC+h)*DV + d] = w_vo[hs][h, d, c*128+rl]
        wvt = np.ascontiguousarray(
            w_vo[hs].transpose(2, 0, 1)               # [512r, HPC, 128d]
            .reshape(NRC, P, HPC * DV).transpose(1, 0, 2)
            .reshape(P, NRC * HPC * DV).astype(bf))
        in_maps.append({
            "qtn": qtn, "qrt": qrt, "kvt": kvt, "krt": krt,
            "wkt": wkt, "wvt": wvt, "mskd": msk,
        })
    return in_maps


def run(q, k, v, w_key, w_vo, trace=False, tmpdir=None):
    """Returns (output [N, H, 128] f32, BassKernelResults)."""
    if "nc" not in _CACHE:
        _CACHE["nc"] = _build()
    nc = _CACHE["nc"]
    in_maps = _prep_inputs(np.asarray(q), np.asarray(k),
                           np.asarray(w_key), np.asarray(w_vo))
    res = run_bass_kernel_spmd(
        nc, in_maps, core_ids=list(range(NCORES)), trace=trace, tmpdir=tmpdir
    )
    outs = [np.asarray(res.results[i]["out"]).astype(np.float32)
            for i in range(NCORES)]
    full = np.concatenate(outs, axis=0)                # [16, 8, 128, 520]
    # device layout [h, qblock, p, j*130 + d] -> token (qblock*512 + j*128 + p)
    full = (full.reshape(H, N // QBLK, P, 4, DVAO).transpose(0, 1, 3, 2, 4)
            .reshape(H, N, DVAO))
    o = full[:, :, :DV] / full[:, :, DV:DV + 1]        # host-side softmax denom
    return np.ascontiguousarray(o.transpose(1, 0, 2)), res


def kernel(q, k, v, w_key, w_vo):
    return run(q, k, v, w_key, w_vo)[0]

